# revision 4
# baseline (speedup 1.0000x reference)
"""LoRA linear (y = x @ (W + s*B@A)^T + bias) on 8 Trainium2 NeuronCores.

Strategy: pure data parallel over the token dim. The LoRA update is folded
into the weight on the host (W' = W + 4.0 * B @ A, rank-8 update), so the
device kernel is a plain linear. x and W' are cast to bf16 on the host
(end-to-end rel fro err ~3e-3, well under the 2e-2 gate); out is written
bf16 and upcast on the host. PSUM accumulation stays fp32.

Per core: out[2048, 1024] = xT[:, shard].T @ wT + bias
  - Engine/queue split (parallel DMA issue, ~650ns per descriptor):
      sync   (HWDGE): x loads, in consumption order
      scalar (HWDGE): w + bias loads, then out stores
      gpsimd:         warmup memset only
      vector:         PSUM eviction (bias add, fp32 psum -> bf16 out tile)
      tensor:         warmup + real matmuls
  - 10 small warmup matmuls (zeroed bf16 scratch) start right after the
    engine preamble so the PE HAM clock-gate window (1.2 -> 2.4 GHz after
    ~3.4us of sustained activity) is mostly paid before real operands land.
  - group 0 (tokens 0-511) runs d-outer/h-middle/i-inner so the matmul
    stream consumes (w[d] half, x[d] quarter) slices in DMA arrival order.
  - groups 1-3 run i-outer/d-inner: each psum tile completes 16 matmuls
    before the next starts, so evictions overlap accumulation and the
    next group's psum reuse never stalls.
  - all matmuls are [128d,128n] bf16 stationary (FWL) x [128d,512o] moving:
    1 col/cycle, same PE rate as fp32r, half the DMA bytes.
"""

import os
import sys

import numpy as np

for _p in ("/opt/trn_rl_repo", "/opt/pypackages"):
    if os.path.isdir(_p) and _p not in sys.path:
        sys.path.append(_p)

try:
    import jax

    jax.config.update(
        "jax_compilation_cache_dir", os.path.expanduser("~/.cache/jax_bass_cache")
    )
    jax.config.update("jax_persistent_cache_min_compile_time_secs", 0.0)
except Exception:
    pass

try:
    # bass_utils imports this when tracing is requested via BASS_TRACE; the
    # agent image ships a stub antenv without it. Register a no-op fallback
    # so a trace request degrades to "no trace" instead of crashing.
    from antenv import axon_hooks as _axon_hooks  # noqa: F401
except ImportError:
    import types as _types

    import antenv as _antenv

    _hooks = _types.ModuleType("antenv.axon_hooks")
    _hooks._hook = None
    _hooks.set_axon_ntff_profile_hook = lambda h: setattr(_hooks, "_hook", h)
    _hooks.get_axon_ntff_profile_hook = lambda: _hooks._hook
    sys.modules["antenv.axon_hooks"] = _hooks
    _antenv.axon_hooks = _hooks

import ml_dtypes  # noqa: E402

import concourse.bass as bass  # noqa: E402,F401
import concourse.mybir as mybir  # noqa: E402
import concourse.tile as tile  # noqa: E402
from concourse import bacc  # noqa: E402
from concourse.bass_utils import run_bass_kernel_spmd  # noqa: E402

N_CORES = 8
N_TOK, D_IN, D_OUT = 16384, 1024, 1024
N_SHARD = N_TOK // N_CORES  # 2048 tokens per core
P = 128
SCALING = 4.0  # alpha / r = 32 / 8
BF16 = ml_dtypes.bfloat16

_CACHE: dict = {}


def build_nc():
    f32 = mybir.dt.float32
    bf16 = mybir.dt.bfloat16
    nc = bacc.Bacc("TRN2", target_bir_lowering=False, debug=False)

    xT = nc.dram_tensor("xT", [D_IN, N_SHARD], bf16, kind="ExternalInput")
    wT = nc.dram_tensor("wT", [D_IN, D_OUT], bf16, kind="ExternalInput")
    bias = nc.dram_tensor("bias", [1, D_OUT], f32, kind="ExternalInput")
    out = nc.dram_tensor("out", [N_SHARD, D_OUT], bf16, kind="ExternalOutput")

    KT = D_IN // P  # 8 contraction tiles
    NBLK = 512  # tokens per group (4 psum tiles of 128)
    GRP = NBLK // P  # 4 psum tiles accumulated concurrently (8 banks)
    OH = 512  # one PSUM bank of fp32 = max moving free dim
    NGRP = N_SHARD // NBLK

    with tile.TileContext(nc) as tc:
        with tc.tile_pool(name="const", bufs=1) as const_pool, \
                tc.tile_pool(name="op", bufs=6) as out_pool, \
                tc.tile_pool(name="ps", bufs=GRP, space="PSUM") as psum_pool:
            w_tiles = [
                const_pool.tile([P, D_OUT], bf16, name=f"w{t}")
                for t in range(KT)
            ]
            x_tiles = [
                const_pool.tile([P, N_SHARD], bf16, name=f"x{t}")
                for t in range(KT)
            ]
            bias_sb = const_pool.tile([P, D_OUT], f32)

            # Warmup: zeroed bf16 scratch matmuls keep the PE busy from the
            # end of the engine preamble until real operands land (~10.1us:
            # begin barrier ~7.0 + descriptor issue + ~2.2us DMA completion
            # latency), so the HAM clock-gate's cold window overlaps the
            # DMA fill. 18 N=256 matmuls at the cold rate span ~3.8us.
            warm = const_pool.tile([P, 256], bf16)
            nc.vector.memset(warm[:], 0.0)
            warm_ps = psum_pool.tile([P, 256], f32, name="warm_ps", tag="psum")
            for _ in range(18):
                nc.tensor.matmul(warm_ps[:], warm[:, 0:P], warm[:],
                                 start=True, stop=True)

            # w + bias on the scalar HWDGE queue, in consumption order.
            # t0/t1 split into o-halves so the h0 slices (consumed first)
            # complete ~0.6us earlier than a full-tile descriptor would.
            for t in range(2):
                for h in range(2):
                    nc.scalar.dma_start(
                        w_tiles[t][:, h * OH:(h + 1) * OH],
                        wT[t * P:(t + 1) * P, h * OH:(h + 1) * OH],
                    )
            for t in range(2, KT):
                nc.scalar.dma_start(w_tiles[t][:], wT[t * P:(t + 1) * P, :])
            nc.scalar.dma_start(bias_sb[:], bias[:].to_broadcast((P, D_OUT)))

            # x on the sync HWDGE queue, in consumption order: all 8 d-tiles
            # of group 0's tokens first (first d-tile split so matmul #1
            # waits on 32KB), then groups 1-3.
            nc.sync.dma_start(x_tiles[0][:, 0:P], xT[0:P, 0:P])
            nc.sync.dma_start(x_tiles[0][:, P:NBLK], xT[0:P, P:NBLK])
            for t in range(1, KT):
                nc.sync.dma_start(x_tiles[t][:, 0:NBLK],
                                  xT[t * P:(t + 1) * P, 0:NBLK])
            for g in range(1, NGRP):
                sl = slice(g * NBLK, (g + 1) * NBLK)
                for t in range(KT):
                    nc.sync.dma_start(x_tiles[t][:, sl],
                                      xT[t * P:(t + 1) * P, sl])

            def evict(g, i, psum, split_store=False):
                n0 = g * NBLK + i * P
                o_sb = out_pool.tile([P, D_OUT], bf16)
                for h in range(2):
                    sl = slice(h * OH, (h + 1) * OH)
                    nc.vector.tensor_add(o_sb[:, sl], psum[:, sl],
                                         bias_sb[:, sl])
                    if split_store:
                        nc.scalar.dma_start(out[n0:n0 + P, sl], o_sb[:, sl])
                if not split_store:
                    nc.scalar.dma_start(out[n0:n0 + P, :], o_sb[:])

            # Group 0: d-outer / h-middle / i-inner — consumes (w[d] half,
            # x[d] group-0 tokens) in exactly the order the DMA queues
            # deliver them, so the PE starts on ~160KB of arrivals. The
            # final d row goes i-outer with immediate eviction so the four
            # psum tiles free up staggered (group 1's psum reuse would
            # otherwise stall ~2us on the eviction backlog).
            psums = [
                psum_pool.tile([P, D_OUT], f32, name=f"ps_g0_{i}", tag="psum")
                for i in range(GRP)
            ]
            for t in range(KT - 1):
                for h in range(2):
                    osl = slice(h * OH, (h + 1) * OH)
                    for i in range(GRP):
                        nc.tensor.matmul(
                            psums[i][:, osl],
                            x_tiles[t][:, i * P:(i + 1) * P],
                            w_tiles[t][:, osl],
                            start=(t == 0),
                            stop=False,
                        )
            t = KT - 1
            for i in range(GRP):
                for h in range(2):
                    osl = slice(h * OH, (h + 1) * OH)
                    nc.tensor.matmul(
                        psums[i][:, osl],
                        x_tiles[t][:, i * P:(i + 1) * P],
                        w_tiles[t][:, osl],
                        start=False,
                        stop=True,
                    )
                evict(0, i, psums[i])

            # Groups 1-3: i-outer / d-inner — each psum tile finishes early
            # in the group, so its eviction (vector) and store (scalar)
            # overlap the next tile's accumulation, and the next group's
            # reuse of the psum slot never waits. The very last tile goes
            # h-outer so after the final matmul only one [128,512] eviction
            # half and one 128KB store remain on the critical tail.
            for g in range(1, NGRP):
                for i in range(GRP):
                    psum = psum_pool.tile([P, D_OUT], f32,
                                          name=f"ps_g{g}_{i}", tag="psum")
                    tok = slice(g * NBLK + i * P, g * NBLK + (i + 1) * P)
                    last = (g == NGRP - 1 and i == GRP - 1)
                    if not last:
                        for t in range(KT):
                            for h in range(2):
                                osl = slice(h * OH, (h + 1) * OH)
                                nc.tensor.matmul(
                                    psum[:, osl],
                                    x_tiles[t][:, tok],
                                    w_tiles[t][:, osl],
                                    start=(t == 0),
                                    stop=(t == KT - 1),
                                )
                        evict(g, i, psum)
                    else:
                        n0 = g * NBLK + i * P
                        o_sb = out_pool.tile([P, D_OUT], bf16)
                        for h in range(2):
                            osl = slice(h * OH, (h + 1) * OH)
                            for t in range(KT):
                                nc.tensor.matmul(
                                    psum[:, osl],
                                    x_tiles[t][:, tok],
                                    w_tiles[t][:, osl],
                                    start=(t == 0),
                                    stop=(t == KT - 1),
                                )
                            nc.vector.tensor_add(o_sb[:, osl], psum[:, osl],
                                                 bias_sb[:, osl])
                            nc.scalar.dma_start(out[n0:n0 + P, osl],
                                                o_sb[:, osl])

    nc.finalize()
    return nc


def _get_nc():
    if "nc" not in _CACHE:
        _CACHE["nc"] = build_nc()
    return _CACHE["nc"]


def kernel(x, weight, bias, A, B):
    x = np.asarray(x, dtype=np.float32)
    weight = np.asarray(weight, dtype=np.float32)
    bias = np.asarray(bias, dtype=np.float32)
    A = np.asarray(A, dtype=np.float32)
    B = np.asarray(B, dtype=np.float32)

    # Fold the rank-8 LoRA update into the weight (exact up to fp32 rounding).
    w_eff = (
        weight.astype(np.float64) + SCALING * (B.astype(np.float64) @ A.astype(np.float64))
    ).astype(np.float32)
    wT = np.ascontiguousarray(w_eff.T.astype(BF16))  # [d, o] bf16
    xT = x.T.astype(BF16)  # [d, n] bf16
    bias2d = np.ascontiguousarray(bias.reshape(1, D_OUT))

    nc = _get_nc()
    in_maps = [
        {
            "xT": np.ascontiguousarray(xT[:, c * N_SHARD:(c + 1) * N_SHARD]),
            "wT": wT,
            "bias": bias2d,
        }
        for c in range(N_CORES)
    ]
    trace_kwargs = {}
    if os.environ.get("KERNEL_TRACE") == "1":
        trace_kwargs = {"trace": True}
    res = run_bass_kernel_spmd(nc, in_maps, list(range(N_CORES)), **trace_kwargs)
    _CACHE["last_results"] = res
    out = np.concatenate([r["out"] for r in res.results], axis=0)
    return out.astype(np.float32)
